# revision 18
# baseline (speedup 1.0000x reference)
"""Trainium2 Bass kernel for CenterWoParamMultiCosineLossV2 (v2).

Math (per sample b with label l, raw scores sc_k = <x_b, centers[l,k]>):
    d_k = 1 + sc_k,  q = sum_k sc_k^2,  u = sum_k sc_k
    value = sum(d^2)/sum(d) = (8 + 2u + q)/(8 + u) = 2 + (q - 8)/(8 + u)
    loss  = mean_b value

Precision: den = 8 + u nearly cancels (min |den| ~ 5.6e-3) and the batch
mean is dominated by those samples -> u needs ~18+ effective bits while q
only needs ~11.  Scheme: x = xh + xl/32 (two fp16 planes); the q path uses
xh only vs an fp16 center table; the u path plays BOTH planes against
fp16 csum-hi/lo columns (22 x 22 bits).

Per-core layout (host packs classes onto cores, <= 12 slots each):
  * ONE stationary table per contraction chunk: cols 0:96 = centers
    (12 slots x 8), 96:120 = csum hi|lo pairs, 120:144 the same /32 for
    the xl pass.  xh pass writes PSUM rows 0:120, xl pass accumulates
    rows 96:120 only -> u rows complete at full precision.
  * Epilogue per 512-sample half: ACT squares rows 0:96 into fp16,
    ACT copies u rows (f32); then per 128-sample block two tiny PE
    matmuls against indicator matrices transpose AND group-reduce in one
    step: q_t = sq_block^T @ Mq [128,12], u_t = u_block^T @ PairSum
    [128,12].  DVE: mask-mul by the one-hot slot mask, reduce, then
    den/num/ratio.  Host sums and adds the constant 2.
  * Startup: DMAs + a DVE memset + PE clock-warmup matmuls are hoisted
    before the tile entry barrier so transfers and the HAM clock ramp
    run during the fixed ~6us engine preamble.
"""

import numpy as np
from contextlib import ExitStack

import concourse.bass as bass
import concourse.tile as tile
import concourse.mybir as mybir
from concourse import bass_utils

_WALRUS_EXTRA_FLAGS = []
_orig_run_command = bass_utils.run_command


def _run_command_flags(argv, **kwargs):
    if (
        _WALRUS_EXTRA_FLAGS
        and isinstance(argv, list)
        and argv
        and "walrus_driver" in str(argv[0])
        and any("codegen" in str(a) for a in argv)
    ):
        argv = list(argv) + _WALRUS_EXTRA_FLAGS
    return _orig_run_command(argv, **kwargs)


bass_utils.run_command = _run_command_flags

# ---------------------------------------------------------------------------
# Workaround: this walrus build accepts only ONE sem-wait per instruction
# ("Too many sync wait commands"), but Tile freely attaches several waits at
# join points.  Post-pass: for any instruction with k>1 waits, hoist k-1 of
# them onto same-engine nops inserted immediately before it.
# ---------------------------------------------------------------------------
_SPLIT_ID = [0]


def _split_multi_waits(nc):
    for f in nc.m.functions:
        for blk in f.blocks:
            insts = blk.instructions
            for idx in range(len(insts) - 1, -1, -1):
                inst = insts[idx]
                si = inst.sync_info
                waits = list(si.on_wait or []) if si is not None else []
                if len(waits) <= 1:
                    continue
                # For DMA instructions, keep a COMPUTE dependency on the
                # instruction (it rides the queue descriptor) and hoist the
                # early-firing queue-guard sems onto the engine nop.
                if type(inst).__name__ == "InstDMACopy":
                    comp = [
                        w
                        for w in waits
                        if not str(w.ant_name or "").startswith("DMA")
                    ]
                    if comp:
                        keep = comp[-1]
                        waits = [w for w in waits if w is not keep] + [keep]
                inst.sync_info = mybir.SyncInfo(
                    on_wait=[waits[-1]], on_update=list(si.on_update or [])
                )
                for w in reversed(waits[:-1]):
                    _SPLIT_ID[0] += 1
                    nop = mybir.InstNoOp(
                        name=f"I-waitsplit-{_SPLIT_ID[0]}", ins=[], outs=[]
                    )
                    nop.engine = inst.engine
                    nop.sync_info = mybir.SyncInfo(on_wait=[w], on_update=[])
                    insts.insert(idx, nop)


def _rewrite_range_clears(nc):
    """Replace EVENT_SEMAPHORE_RANGE_CLEAR raw-ISA (rejected by this walrus
    build) with per-sem InstEventSemaphore writes on the same engine."""
    import re

    for f in nc.m.functions:
        for blk in f.blocks:
            insts = blk.instructions
            for idx in range(len(insts) - 1, -1, -1):
                inst = insts[idx]
                if type(inst).__name__ != "InstISA":
                    continue
                s = str(inst)
                if "EVENT_SEMAPHORE_RANGE_CLEAR" not in s:
                    continue
                first = int(re.search(r"range_first=(\d+)", s).group(1))
                last = int(re.search(r"range_last=(\d+)", s).group(1))
                si = inst.sync_info
                waits = list(si.on_wait or []) if si is not None else []
                upds = list(si.on_update or []) if si is not None else []
                repl = []
                for j, sem in enumerate(range(first, last + 1)):
                    _SPLIT_ID[0] += 1
                    ev = mybir.InstEventSemaphore(
                        name=f"I-semclr-{_SPLIT_ID[0]}", ins=[], outs=[]
                    )
                    ev.engine = inst.engine
                    ev.sync_info = mybir.SyncInfo(
                        on_wait=waits if j == 0 else [],
                        on_update=[
                            mybir.SyncUpdate(
                                sync_type="semaphore",
                                id=sem,
                                update_mode="sem-wr-imm",
                                update_value=0,
                            )
                        ]
                        + (upds if j == (last - first) else []),
                    )
                    repl.append(ev)
                insts[idx : idx + 1] = repl


def _trim_tail(nc):
    """Delete the redundant TileContext tail sem-clears + second barrier
    (the walrus codegen epilogue already clears every semaphore)."""
    f = nc.m.functions[0]
    blocks = {b.name: b for b in f.blocks}
    end = [b for n, b in blocks.items() if n.endswith("_end")][0]

    insts = end.instructions
    clr_idx = [i for i, x in enumerate(insts) if x.name.startswith("I-semclr-")]
    if not clr_idx:
        return
    start_del = clr_idx[0]
    if start_del > 0 and type(insts[start_del - 1]).__name__ == "InstDrain":
        start_del -= 1
    del insts[start_del:]


H_SEM = 206  # handshake sem: free, cleared late in DVE's walrus-epilogue run


def _relax_end_barrier(nc):
    """Replace the end-of-tile all-engine barrier with a minimal handshake
    so engines fall through to their walrus sem-clear partitions as soon as
    their own work ends (saves the serial ~6us clear tail)."""
    f = nc.m.functions[0]
    for blk in f.blocks:
        for inst in blk.instructions:
            si = inst.sync_info
            if si is not None:
                assert all(w.id != H_SEM for w in (si.on_wait or [])), "H_SEM in use"
                assert all(u.id != H_SEM for u in (si.on_update or [])), "H_SEM in use"
    blocks = {b.name: b for b in f.blocks}
    end = [b for n, b in blocks.items() if n.endswith("_end")][0]

    barrier_ids = {151, 152}
    keep = []
    for inst in end.instructions:
        si = inst.sync_info
        refs = set()
        if si is not None:
            refs |= {w.id for w in (si.on_wait or [])}
            refs |= {u.id for u in (si.on_update or [])}
        tn = type(inst).__name__
        if tn in ("InstDrain", "InstEventSemaphore") and (
            (refs and refs <= barrier_ids) or not refs
        ):
            continue  # barrier choreography / bare engine drains
        keep.append(inst)
    end.instructions[:] = keep

    # DVE & Pool park on H at the head of the end block
    for eng in (mybir.EngineType.DVE, mybir.EngineType.Pool):
        _SPLIT_ID[0] += 1
        nop = mybir.InstNoOp(name=f"I-relaxwait-{_SPLIT_ID[0]}", ins=[], outs=[])
        nop.engine = eng
        nop.sync_info = mybir.SyncInfo(
            on_wait=[
                mybir.SyncWait(
                    sync_type="semaphore",
                    id=H_SEM,
                    wait_mode="sem-ge-imm",
                    wait_value=1,
                )
            ],
            on_update=[],
        )
        end.instructions.insert(0, nop)

    # SP incs H right after the last DMA issue in the tile body
    tile_blocks = [
        b for b in f.blocks
        if not b.name.endswith("_end") and b.name != "main"
    ]
    last_dma = None
    for b in tile_blocks:
        for i, inst in enumerate(b.instructions):
            if type(inst).__name__ == "InstDMACopy":
                last_dma = (b, i)
    assert last_dma is not None
    b, i = last_dma
    _SPLIT_ID[0] += 1
    inc = mybir.InstEventSemaphore(name=f"I-relaxinc-{_SPLIT_ID[0]}", ins=[], outs=[])
    inc.engine = b.instructions[i].engine
    inc.sync_info = mybir.SyncInfo(
        on_wait=[],
        on_update=[
            mybir.SyncUpdate(
                sync_type="semaphore",
                id=H_SEM,
                update_mode="sem-inc",
                update_value=1,
            )
        ],
    )
    b.instructions.insert(i + 1, inc)


def _hoist_all(nc):
    """Move the ENTIRE tile body into main BEFORE the all-engine entry
    barrier.  All cross-engine ordering is carried by the tile-emitted
    sems, which come along -- the barrier only separated the framework
    const memsets from the body, and the only pre-barrier consumer (the
    dummy table-load Square) tolerates garbage.  This stops slow engines
    (PE warmups, DMA issue) from delaying every OTHER engine's body via
    the barrier."""
    f = nc.m.functions[0]
    blocks = {b.name: b for b in f.blocks}
    main = blocks["main"]
    tile_blocks = [
        b for b in f.blocks
        if not b.name.endswith("_end") and b.name != "main"
    ]
    moved = []
    for b in tile_blocks:
        moved.extend(b.instructions)
        b.instructions[:] = []
    m_insts = main.instructions
    ins_pt = next(
        (i for i, x in enumerate(m_insts) if type(x).__name__ == "InstDrain"),
        len(m_insts),
    )
    for j, inst in enumerate(moved):
        m_insts.insert(ins_pt + j, inst)


# ---------------------------------------------------------------------------

B, D, NCLS, KC = 8192, 512, 90, 8
NCORES, P = 8, 128
BC = B // NCORES          # samples per core
NBLK = BC // P            # 128-sample blocks per core
KCH = D // P              # contraction chunks
NTILE = 512               # moving-operand columns per matmul
NH = BC // NTILE          # 512-sample halves per core
NB = NTILE // P           # 128-sample blocks per half
NSLOT = 12                # class slots per core
TC = NSLOT * KC           # 96 center columns (PSUM rows 0:96)
UC = 2 * NSLOT            # 24 csum hi|lo rows (PSUM rows 96:120)
UOFF = TC                 # base row of the xh-pass u rows
NSEL = NSLOT + 1          # 13 select cols: 12 slots + bias col (+-8 consts)
ULOFF = 32                # ul32 rows base (32-aligned partition access)
U2W = ULOFF + UC + 1      # 57 u-select rows: uh32 | pad | ul32 | bias
TW = 168                  # 0:128 stationary | 128:152 U2/32 | 152:165 Mq | pad
NWARM = 40                # PE clock-warmup matmuls

_BUILD_CACHE = {}


def _build(post_process=True):
    f32 = mybir.dt.float32
    f16 = mybir.dt.float16
    nc = bass.Bass("TRN2", target_bir_lowering=False, debug=False, num_devices=1)
    xh_d = nc.dram_tensor("xh", [NH, P, KCH, NTILE], f16, kind="ExternalInput")
    xl_d = nc.dram_tensor("xl", [NH, P, KCH, NTILE], f16, kind="ExternalInput")
    t_d = nc.dram_tensor("t", [P, KCH, TW], f16, kind="ExternalInput")
    ps2_d = nc.dram_tensor("ps2", [U2W, NSEL], f16, kind="ExternalInput")
    e2_d = nc.dram_tensor("e2", [P, NBLK, 2 * NSEL], f32, kind="ExternalInput")
    val_d = nc.dram_tensor("val", [P, NH, NB], f32, kind="ExternalOutput")

    with tile.TileContext(nc) as tc:
        with ExitStack() as ctx:
            consts = ctx.enter_context(tc.tile_pool(name="consts", bufs=1))
            work = ctx.enter_context(tc.tile_pool(name="work", bufs=1))
            pst = ctx.enter_context(tc.tile_pool(name="pst", bufs=2, space="PSUM"))
            pqt = ctx.enter_context(tc.tile_pool(name="pqt", bufs=1, space="PSUM"))
            pwu = ctx.enter_context(tc.tile_pool(name="pwu", bufs=1, space="PSUM"))

            # ---- pre-barrier set: memsets, input DMAs, PE clock warmups
            wu_src = consts.tile([P, NTILE], f16)
            dummy = consts.tile([32, 1], f16)
            st_sq = work.tile([P, NH, NTILE], f16)  # rows 0:32,64:128 sq;
                                                    # row 32: bias = 1
            u2_sb = work.tile([U2W, NH, NTILE], f16)      # row 56: bias = 1
            nc.vector.memset(wu_src, 0.0)
            # bias rows: whole-tile memsets; later writes overwrite the
            # live rows, leaving the bias rows at 1.0
            nc.vector.memset(st_sq, 1.0)
            nc.vector.memset(u2_sb, 1.0)
            # tiny Square to trigger the ~1.3us ACT table load early
            nc.scalar.activation(
                dummy, wu_src[0:32, 0:1],
                mybir.ActivationFunctionType.Square,
            )

            t_sb = consts.tile([P, KCH, TW], f16)
            ps2_sb = consts.tile([U2W, NSEL], f16)
            e2_sb = consts.tile([P, NBLK, 2 * NSEL], f32)
            xh_sb = consts.tile([P, NH, KCH, NTILE], f16)
            xl_sb = consts.tile([P, NH, KCH, NTILE], f16)

            # SP ring: xh0, xh1 (split), ps2   ACT ring: t, xl0, xl1
            # (split), e2.  The half-1 planes are split by chunk pair so
            # their completion sems fire earlier; slow odd-shaped
            # transfers go last.
            nc.sync.dma_start(out=xh_sb[:, 0], in_=xh_d.ap()[0])
            nc.scalar.dma_start(out=t_sb, in_=t_d.ap())
            nc.sync.dma_start(out=xh_sb[:, 1], in_=xh_d.ap()[1])
            nc.scalar.dma_start(out=xl_sb[:, 0], in_=xl_d.ap()[0])
            nc.sync.dma_start(out=ps2_sb, in_=ps2_d.ap())
            nc.scalar.dma_start(out=xl_sb[:, 1], in_=xl_d.ap()[1])
            nc.scalar.dma_start(out=e2_sb, in_=e2_d.ap())

            # HAM clock warm-up: garbage matmuls sized to span the preamble
            # +DMA window so the PE clock is at 2.4 GHz for the real work.
            wu_ps = pwu.tile([P, 256], f32)
            for w in range(NWARM):
                nc.tensor.matmul(
                    wu_ps[0:32, 0:128], wu_src[:, 0:32], wu_src[:, 0:128],
                    start=True, stop=True, skip_group_check=True,
                )

            # ---- tile body
            st_ps = [pst.tile([P, NTILE], f32, name=f"st{n}") for n in range(NH)]
            qt_ps = pqt.tile([P, NH, NB, 2 * NSEL], f32)
            m = work.tile([P, NH, NB, 2 * NSEL], f32)
            colq = work.tile([P, NH, NB], f32)
            den = work.tile([P, NH, NB], f32)
            rde = work.tile([P, NH, NB], f32)
            val = work.tile([P, NH, NB], f32)

            mq = t_sb[0:P, 0, 152 : 152 + NSEL]  # [128,13] fp16

            pin = [0.0]

            def phase():
                pin[0] += 0.001
                return tc.tile_wait_until(pin[0])

            def hx(n):
                # xh vs [U2 | pad | centers]: full 128 PSUM rows
                with phase():
                    for k in range(KCH):
                        nc.tensor.matmul(
                            st_ps[n], t_sb[:, k, 0:P], xh_sb[:, n, k],
                            start=(k == 0), stop=False,
                            skip_group_check=True,
                        )

            def sq(n):
                # squares of the center rows (32:64, 64:128 -- legal bases)
                with phase():
                    nc.scalar.activation(
                        st_sq[0:32, n], st_ps[n][32:64],
                        mybir.ActivationFunctionType.Square, scale=0.5,
                    )
                    nc.scalar.activation(
                        st_sq[64:P, n], st_ps[n][64:P],
                        mybir.ActivationFunctionType.Square, scale=0.5,
                    )

            def hl(n):
                # xl vs U2/32 accumulates into the same u rows (0:24)
                with phase():
                    for k in range(KCH):
                        nc.tensor.matmul(
                            st_ps[n][0:UC], t_sb[:, k, P : P + UC],
                            xl_sb[:, n, k],
                            start=False, stop=(k == KCH - 1),
                            skip_group_check=True,
                        )

            def qsel(n):
                with phase():
                    for j in range(NB):
                        nc.tensor.matmul(
                            qt_ps[:, n, j, 0:NSEL],
                            st_sq[:, n, j * P : (j + 1) * P],
                            mq,
                            start=True, stop=True, skip_group_check=True,
                        )

            def uhi(n):
                # u hi plane on DVE: f16(32*u)
                with phase():
                    nc.vector.tensor_scalar_mul(
                        u2_sb[0:UC, n], st_ps[n][0:UC], 32.0
                    )

            def ulo(n):
                # u lo plane on DVE: 32*u - uh32 (exact residual)
                with phase():
                    nc.vector.scalar_tensor_tensor(
                        u2_sb[ULOFF : ULOFF + UC, n], st_ps[n][0:UC], 32.0,
                        u2_sb[0:UC, n],
                        op0=mybir.AluOpType.mult,
                        op1=mybir.AluOpType.subtract,
                    )

            def usel(n):
                with phase():
                    for j in range(NB):
                        nc.tensor.matmul(
                            qt_ps[:, n, j, NSEL:],
                            u2_sb[:, n, j * P : (j + 1) * P],
                            ps2_sb,
                            start=True, stop=True, skip_group_check=True,
                        )

            def ratio(n):
                bs = slice(n * NB, (n + 1) * NB)
                with phase():
                    nc.vector.tensor_mul(m[:, n], qt_ps[:, n], e2_sb[:, bs])
                    nc.vector.reduce_sum(
                        den[:, n], m[:, n, :, NSEL:], axis=mybir.AxisListType.X
                    )
                    nc.vector.reciprocal(rde[:, n], den[:, n])
                    nc.vector.reduce_sum(
                        colq[:, n], m[:, n, :, 0:NSEL],
                        axis=mybir.AxisListType.X,
                    )
                    nc.vector.tensor_mul(val[:, n], colq[:, n], rde[:, n])

            hx(0)
            hl(0)
            sq(0)
            uhi(0)
            ulo(0)
            hx(1)
            qsel(0)
            hl(1)
            usel(0)
            sq(1)
            uhi(1)
            ulo(1)
            qsel(1)
            ratio(0)
            usel(1)
            ratio(1)
            with phase():
                nc.sync.dma_start(out=val_d.ap(), in_=val)
    if post_process:
        _rewrite_range_clears(nc)
        _trim_tail(nc)
        _relax_end_barrier(nc)
        _hoist_all(nc)
        _split_multi_waits(nc)
    return nc


def _pack_cores(labels):
    """Assign samples to cores: exactly BC samples each, <= NSLOT distinct
    classes each.  Whole-class LPT + randomized restarts; classes at the
    boundary are split across cores."""
    cnt = np.bincount(labels, minlength=NCLS)
    present = [int(c) for c in np.where(cnt > 0)[0]]
    rng = np.random.default_rng(0)
    for trial in range(4000):
        if trial == 0:
            order = sorted(present, key=lambda c: -cnt[c])
        else:
            order = list(rng.permutation(present))
        loads = [0] * NCORES
        groups = [[] for _ in range(NCORES)]
        for c in order:
            i = min(range(NCORES), key=lambda t: loads[t])
            groups[i].append(c)
            loads[i] += int(cnt[c])
        amounts = [
            {c: int(cnt[c]) for c in groups[i]} for i in range(NCORES)
        ]
        ok = True
        for _ in range(64):
            over = [i for i in range(NCORES) if loads[i] > BC]
            if not over:
                break
            i = max(over, key=lambda t: loads[t])
            under = [j for j in range(NCORES) if loads[j] < BC]
            if not under:
                ok = False
                break
            j = max(under, key=lambda t: BC - loads[t])
            amt = min(loads[i] - BC, BC - loads[j])
            c = max(amounts[i], key=lambda t: amounts[i][t])
            amt = min(amt, amounts[i][c])
            amounts[i][c] -= amt
            if amounts[i][c] == 0:
                del amounts[i][c]
            amounts[j][c] = amounts[j].get(c, 0) + amt
            loads[i] -= amt
            loads[j] += amt
        else:
            ok = False
        if not ok:
            continue
        if all(loads[i] == BC for i in range(NCORES)) and all(
            len(amounts[i]) <= NSLOT for i in range(NCORES)
        ):
            return amounts
    raise RuntimeError("could not pack classes into 12 slots per core")


def _prep_in_maps(x, centers, labels):
    x = np.ascontiguousarray(np.asarray(x, dtype=np.float32))
    centers = np.asarray(centers, dtype=np.float32).astype(np.float64)
    labels = np.asarray(labels).astype(np.int64)

    xh_f = x.astype(np.float16)
    xl_f = ((x.astype(np.float64) - xh_f.astype(np.float64)) * 32.0).astype(
        np.float16
    )

    csum = centers.sum(axis=1)                      # [NCLS, D] fp64
    chi_c = centers.astype(np.float16)              # center rows fp16
    cs_hi = csum.astype(np.float16)
    cs_lo = (csum - cs_hi.astype(np.float64)).astype(np.float16)
    # /32 on an fp16 value is an exact exponent shift (modulo subnormals,
    # which are negligible here)
    cs_hi32 = (cs_hi.astype(np.float64) / 32.0).astype(np.float16)
    cs_lo32 = (cs_lo.astype(np.float64) / 32.0).astype(np.float16)

    amounts = _pack_cores(labels)
    pools = {c: list(np.where(labels == c)[0]) for c in range(NCLS)}
    ptr = {c: 0 for c in range(NCLS)}

    in_maps = []
    for i in range(NCORES):
        cls = sorted(amounts[i])
        idx = []
        slot_ids = []
        for s, c in enumerate(cls):
            n = amounts[i][c]
            take = pools[c][ptr[c] : ptr[c] + n]
            ptr[c] += n
            idx.extend(take)
            slot_ids.extend([s] * n)
        idx = np.asarray(idx)
        slot_ids = np.asarray(slot_ids)
        assert len(idx) == BC

        def chunked(plane):
            # [NH, P, KCH, NTILE]: per (half, partition) one 4 KiB read
            return np.ascontiguousarray(
                plane[idx].T.reshape(KCH, P, NH, NTILE).transpose(2, 1, 0, 3)
            )

        xh = chunked(xh_f)
        xl = chunked(xl_f)

        # stationary table [D, TW]: cols 0:24 = csum hi|lo pairs,
        # 24:32 zero pad, 32:128 = centers, 128:152 = csum pairs /32 (xl
        # pass), 152:165 = Mq select matrix (chunk-0 rows 0:97)
        t = np.zeros((D, TW), np.float16)
        for s, c in enumerate(cls):
            t[:, 2 * s] = cs_hi[c]
            t[:, 2 * s + 1] = cs_lo[c]
            t[:, 32 + s * KC : 32 + (s + 1) * KC] = chi_c[c].T
            t[:, P + 2 * s] = cs_hi32[c]
            t[:, P + 2 * s + 1] = cs_lo32[c]
        # Mq rows = st_sq rows (slots 0-3 at 0:32, slots 4-11 at 64:128,
        # matching the two Square regions): 4.0 undoes the 0.5 pre-scale
        # inside Square; bias row 32 carries the -8 constant
        for s in range(NSLOT):
            for j in range(KC):
                r = s * KC + j if s < 4 else 32 + s * KC + j
                t[r, 152 + s] = 4.0
        t[32, 152 + NSLOT] = -8.0
        t = np.ascontiguousarray(t.reshape(KCH, P, TW).transpose(1, 0, 2))

        # u-select matrix: rows 0:24 = uh32, 32:56 = ul32 (both 1/32),
        # bias row 56 -> den's +8; pad rows multiply 0
        ps2 = np.zeros((U2W, NSEL), np.float16)
        for s in range(NSLOT):
            for r in (2 * s, 2 * s + 1):
                ps2[r, s] = 1.0 / 32.0
                ps2[ULOFF + r, s] = 1.0 / 32.0
        ps2[ULOFF + UC, NSLOT] = 8.0

        e = np.zeros((BC, NSEL), np.float32)
        e[np.arange(BC), slot_ids] = 1.0
        e[:, NSLOT] = 1.0  # bias col always selected
        # [P, NBLK, 2*NSEL]: sample (blk*128 + p) -> partition p, block blk
        e2 = np.ascontiguousarray(
            np.broadcast_to(e[:, None, :], (BC, 2, NSEL))
            .reshape(NBLK, P, 2 * NSEL)
            .transpose(1, 0, 2)
            .astype(np.float32)
        )
        in_maps.append({"xh": xh, "xl": xl, "t": t, "ps2": ps2, "e2": e2})
    return in_maps


def kernel(x, centers, labels, _trace=False):
    in_maps = _prep_in_maps(x, centers, labels)
    if "nc" not in _BUILD_CACHE:
        _BUILD_CACHE["nc"] = _build()
    nc = _BUILD_CACHE["nc"]
    res = bass_utils.run_bass_kernel_spmd(
        nc, in_maps, core_ids=list(range(NCORES)), trace=_trace
    )
    total = 0.0
    for r in res.results:
        total += r["val"].astype(np.float64).sum()
    out = np.float32(2.0 + total / B)
    if _trace:
        return out, res
    return out
